# Initial kernel scaffold
#
"""KNN (k=16) over B=2, N=8192, D=3 points on 8 TRN2 NeuronCores.

Strategy
--------
Shard the 2*8192 queries across 8 cores (batch b = core//4, query chunk
core%4 of 2048 queries). Every core holds the full 8192 keys of its batch.

Per core, for each 128-query block:
  - distances via one K=4 augmented matmul per 512-key chunk:
        t(n, m) = 2*q_n . k_m - ||k_m||^2   (maximize t  <=>  minimize d2)
    lhsT = [qx; qy; qz; 1] (stationary), rhs = [2kx; 2ky; 2kz; -||k||^2]
  - ACT copies PSUM->SBUF row tile [128, 8192]
  - DVE: per-chunk max8 -> V[128, 16*8], then top-16 of V, then two
    max_index passes over the full row to recover the key indices.
Indices come out rank-ordered (ascending squared distance), ties by lower
index — matching jax.lax.top_k on -d2.
"""

import numpy as np

B = 2
N = 8192
K = 16
N_CORES = 8
QPC = (B * N) // N_CORES  # queries per core: 2048
QB = 128                  # query block (partition dim)
KC = 512                  # key chunk (one PSUM bank of f32)
N_QB = QPC // QB          # 16
N_KC = N // KC            # 16
NEG_BIG = -1.0e30

_cached = {}


def _build_nc():
    import concourse.bass as bass
    import concourse.mybir as mybir
    from concourse import tile

    f32 = mybir.dt.float32
    u32 = mybir.dt.uint32

    nc = bass.Bass()
    aq = nc.declare_dram_parameter("aq", [4, QPC], f32, isOutput=False)
    ak = nc.declare_dram_parameter("ak", [4, N], f32, isOutput=False)
    out = nc.declare_dram_parameter("out", [QPC, K], u32, isOutput=True)

    with tile.TileContext(nc) as tc:
        with (
            tc.tile_pool(name="const", bufs=1) as cpool,
            tc.tile_pool(name="psum", bufs=4, space="PSUM") as ppool,
            tc.tile_pool(name="rows", bufs=2) as rpool,
            tc.tile_pool(name="small", bufs=2) as spool,
        ):
            aq_sb = cpool.tile([4, QPC], f32, tag="aq", name="aq_sb")
            nc.sync.dma_start(out=aq_sb[:], in_=aq[:])
            ak_sb = cpool.tile([4, N], f32, tag="ak", name="ak_sb")
            nc.sync.dma_start(out=ak_sb[:], in_=ak[:])

            for qb in range(N_QB):
                row = rpool.tile([QB, N], f32, tag="row", name="row")
                V = spool.tile([QB, 8 * N_KC], f32, tag="V", name="V")
                for kc in range(N_KC):
                    ps = ppool.tile([QB, KC], f32, tag="ps", name="ps")
                    nc.tensor.matmul(
                        ps[:],
                        lhsT=aq_sb[:, qb * QB:(qb + 1) * QB],
                        rhs=ak_sb[:, kc * KC:(kc + 1) * KC],
                        start=True,
                        stop=True,
                    )
                    nc.scalar.activation(
                        row[:, kc * KC:(kc + 1) * KC],
                        ps[:],
                        mybir.ActivationFunctionType.Copy,
                    )
                    nc.vector.max(V[:, kc * 8:(kc + 1) * 8],
                                  row[:, kc * KC:(kc + 1) * KC])

                a8 = spool.tile([QB, 8], f32, tag="a8", name="a8")
                b8 = spool.tile([QB, 8], f32, tag="b8", name="b8")
                ia = spool.tile([QB, 8], u32, tag="ia", name="ia")
                ib = spool.tile([QB, 8], u32, tag="ib", name="ib")

                nc.vector.max(a8[:], V[:])
                nc.vector.max_index(ia[:], a8[:], row[:])
                nc.vector.match_replace(V[:], a8[:], V[:], NEG_BIG)
                nc.vector.max(b8[:], V[:])
                nc.vector.max_index(ib[:], b8[:], row[:])

                nc.sync.dma_start(out=out[qb * QB:(qb + 1) * QB, 0:8], in_=ia[:])
                nc.sync.dma_start(out=out[qb * QB:(qb + 1) * QB, 8:16], in_=ib[:])
    return nc


def _get_nc():
    if "nc" not in _cached:
        _cached["nc"] = _build_nc()
    return _cached["nc"]


def _make_in_maps(points):
    pts = np.ascontiguousarray(np.asarray(points, dtype=np.float32))
    assert pts.shape == (B, N, 3), pts.shape
    sq = (pts * pts).sum(axis=-1, dtype=np.float32)  # (B, N)
    in_maps = []
    for c in range(N_CORES):
        b = c // (N_CORES // B)
        qc = c % (N_CORES // B)
        q = pts[b, qc * QPC:(qc + 1) * QPC, :]  # (QPC, 3)
        aq = np.empty((4, QPC), dtype=np.float32)
        aq[0:3, :] = q.T
        aq[3, :] = 1.0
        ak = np.empty((4, N), dtype=np.float32)
        ak[0:3, :] = (2.0 * pts[b]).T
        ak[3, :] = -sq[b]
        in_maps.append({"aq": np.ascontiguousarray(aq),
                        "ak": np.ascontiguousarray(ak)})
    return in_maps


def run(points, k, trace=False):
    from concourse.bass_utils import run_bass_kernel_spmd

    assert int(k) == K
    nc = _get_nc()
    in_maps = _make_in_maps(points)
    res = run_bass_kernel_spmd(nc, in_maps, core_ids=list(range(N_CORES)),
                               trace=trace)
    idx = np.empty((B, N, K), dtype=np.int32)
    for c in range(N_CORES):
        b = c // (N_CORES // B)
        qc = c % (N_CORES // B)
        o = np.asarray(res.results[c]["out"]).astype(np.int64)
        idx[b, qc * QPC:(qc + 1) * QPC, :] = o.astype(np.int32)
    return idx, res


def kernel(points, k):
    idx, _ = run(points, k, trace=False)
    return idx


# revision 11
# speedup vs baseline: 1.9580x; 1.9580x over previous
"""KNN (k=16) over B=2, N=8192, D=3 points on 8 TRN2 NeuronCores.

Strategy
--------
Shard the 2*8192 queries across 8 cores (batch b = core//4, query chunk
core%4 of 2048 queries). Every core holds the full 8192 keys of its batch.

The reference (jax on the neuron backend) computes
    d2 = (sq_n + sq_m) - 2*einsum(q, k)
and at full size its einsum lowers to PE matmuls with the QUERIES as
the stationary operand (verified bit-for-bit).  To be bit-exact (the
rel-err gate on integer indices punishes any near-tie reordering), we
replicate the arithmetic exactly:
  - PE matmul, queries stationary: psum[q128, k512] = inner
  - ACT copy with scale=2.0:       row_raw = 2*inner (exact doubling)
  - ACT Identity+bias:             row = sqk + sq_n  (one IEEE add,
                                   same association as the reference)
  - GPSIMD tensor_sub:             row = row_raw - row = -(d2) bitwise
Top-16 per row of -d2 (descending) via DVE max8 / match_replace /
max_index — ascending squared distance, ties resolved like jax.lax.top_k
except ties straddling the rank-8/9 boundary (rare; ~2e-3 of rows).
"""

import numpy as np

B = 2
N = 8192
K = 16
N_CORES = 8
QPC = (B * N) // N_CORES  # queries per core: 2048
QB = 128                  # query block (partition dim)
KC = 512                  # key chunk for DVE segmented max8
N_QB = QPC // QB          # 16
N_KC = N // KC            # 16
NEG_BIG = -1.0e30

_cached = {}


def _build_nc():
    import concourse.mybir as mybir
    from concourse import bacc, tile

    f32 = mybir.dt.float32
    u32 = mybir.dt.uint32
    Copy = mybir.ActivationFunctionType.Copy

    Identity = mybir.ActivationFunctionType.Identity

    nc = bacc.Bacc()
    qT = nc.declare_dram_parameter("qT", [3, QPC], f32, isOutput=False)
    kT = nc.declare_dram_parameter("kT", [3, N], f32, isOutput=False)
    sqq = nc.declare_dram_parameter("sqq", [QB, N_QB], f32, isOutput=False)
    sqk = nc.declare_dram_parameter("sqk", [1, N], f32, isOutput=False)
    out = nc.declare_dram_parameter("out", [QPC, K], u32, isOutput=True)

    with tile.TileContext(nc) as tc:
        with (
            tc.tile_pool(name="const", bufs=1) as cpool,
            tc.tile_pool(name="mm", bufs=4, space="PSUM") as mmpool,
            tc.tile_pool(name="ch", bufs=4) as chpool,
            tc.tile_pool(name="rows", bufs=2) as rpool,
            tc.tile_pool(name="small", bufs=2) as spool,
        ):
            qT_sb = cpool.tile([3, QPC], f32, tag="qT", name="qT_sb")
            nc.sync.dma_start(out=qT_sb[:], in_=qT[:])
            kT_sb = cpool.tile([3, N], f32, tag="kT", name="kT_sb")
            nc.sync.dma_start(out=kT_sb[:], in_=kT[:])
            sqq_sb = cpool.tile([QB, N_QB], f32, tag="sqq", name="sqq_sb")
            nc.sync.dma_start(out=sqq_sb[:], in_=sqq[:])
            sqk_sb = cpool.tile([QB, N], f32, tag="sqk", name="sqk_sb")
            nc.sync.dma_start(out=sqk_sb[:], in_=sqk[0:1, :].partition_broadcast(QB))

            for qb in range(N_QB):
                row = rpool.tile([QB, N], f32, tag="row", name="row")
                V = spool.tile([QB, 8 * N_KC], f32, tag="V", name="V")

                for kc in range(N_KC):
                    sl = slice(kc * KC, (kc + 1) * KC)
                    ps_mm = mmpool.tile([QB, KC], f32, tag="ps_mm", name="ps_mm")
                    nc.tensor.matmul(
                        ps_mm[:],
                        lhsT=qT_sb[:, qb * QB:(qb + 1) * QB],
                        rhs=kT_sb[:, sl],
                        start=True,
                        stop=True,
                    )
                    # ch = 2*inner (exact doubling)
                    ch = chpool.tile([QB, KC], f32, tag="ch", name="ch")
                    nc.scalar.activation(ch[:], ps_mm[:], Copy, scale=2.0)
                    # row = sqk + sq_n  (the reference's (sq_n + sq_m) add)
                    nc.scalar.activation(
                        row[:, sl], sqk_sb[:, sl], Identity,
                        bias=sqq_sb[:, qb:qb + 1], scale=1.0)
                    # row = 2*inner - (sq_n+sq_m) = -d2 bitwise
                    nc.gpsimd.tensor_sub(row[:, sl], ch[:], row[:, sl])
                    nc.vector.max(V[:, kc * 8:(kc + 1) * 8], row[:, sl])

                a8 = spool.tile([QB, 8], f32, tag="a8", name="a8")
                b8 = spool.tile([QB, 8], f32, tag="b8", name="b8")
                ia = spool.tile([QB, 8], u32, tag="ia", name="ia")
                ib = spool.tile([QB, 8], u32, tag="ib", name="ib")

                nc.vector.max(a8[:], V[:])
                nc.vector.max_index(ia[:], a8[:], row[:])
                nc.vector.match_replace(V[:], a8[:], V[:], NEG_BIG)
                nc.vector.max(b8[:], V[:])
                nc.vector.max_index(ib[:], b8[:], row[:])

                nc.sync.dma_start(out=out[qb * QB:(qb + 1) * QB, 0:8], in_=ia[:])
                nc.sync.dma_start(out=out[qb * QB:(qb + 1) * QB, 8:16], in_=ib[:])
    nc.compile()
    return nc


def _get_nc():
    if "nc" not in _cached:
        _cached["nc"] = _build_nc()
    return _cached["nc"]


def _make_in_maps(points):
    pts = np.ascontiguousarray(np.asarray(points, dtype=np.float32))
    assert pts.shape == (B, N, 3), pts.shape
    # sq exactly like the reference computes it on device: sequential f32
    sq = ((pts[..., 0] * pts[..., 0] + pts[..., 1] * pts[..., 1])
          + pts[..., 2] * pts[..., 2]).astype(np.float32)
    in_maps = []
    for c in range(N_CORES):
        b = c // (N_CORES // B)
        qc = c % (N_CORES // B)
        q = pts[b, qc * QPC:(qc + 1) * QPC, :]
        sqq = sq[b, qc * QPC:(qc + 1) * QPC]
        in_maps.append({
            "qT": np.ascontiguousarray(q.T),
            "kT": np.ascontiguousarray(pts[b].T),
            "sqq": np.ascontiguousarray(sqq.reshape(N_QB, QB).T),
            "sqk": np.ascontiguousarray(sq[b][None, :]),
        })
    return in_maps


def _make_runner(nc, n_cores):
    """Build a cached jitted SPMD executor for ``nc`` (axon PJRT path).

    Mirrors concourse.bass2jax.run_bass_via_pjrt but caches the jitted
    callable so repeated calls don't re-trace/re-compile.
    """
    import jax
    import numpy as _np
    from jax.sharding import Mesh, PartitionSpec
    try:
        from jax.experimental.shard_map import shard_map
    except ImportError:
        from jax.sharding import shard_map  # newer jax
    import concourse.mybir as mybir
    from concourse.bass2jax import (_bass_exec_p, install_neuronx_cc_hook,
                                    partition_id_tensor)

    install_neuronx_cc_hook()

    partition_name = (nc.partition_id_tensor.name
                      if nc.partition_id_tensor else None)
    in_names, out_names, out_avals, zero_outs = [], [], [], []
    for alloc in nc.m.functions[0].allocations:
        if not isinstance(alloc, mybir.MemoryLocationSet):
            continue
        name = alloc.memorylocations[0].name
        if alloc.kind == "ExternalInput":
            if name != partition_name:
                in_names.append(name)
        elif alloc.kind == "ExternalOutput":
            out_names.append(name)
            shape = tuple(alloc.tensor_shape)
            dtype = mybir.dt.np(alloc.dtype)
            out_avals.append(jax.core.ShapedArray(shape, dtype))
            zero_outs.append(_np.zeros(shape, dtype))
    n_params = len(in_names)
    n_outs = len(out_avals)
    all_in_names = list(in_names) + list(out_names)
    if partition_name is not None:
        all_in_names.append(partition_name)
    donate = tuple(range(n_params, n_params + n_outs))

    def _body(*args):
        operands = list(args)
        if partition_name is not None:
            operands.append(partition_id_tensor())
        outs = _bass_exec_p.bind(
            *operands,
            out_avals=tuple(out_avals),
            in_names=tuple(all_in_names),
            out_names=tuple(out_names),
            lowering_input_output_aliases=(),
            sim_require_finite=True,
            sim_require_nnan=True,
            nc=nc,
        )
        return tuple(outs)

    devices = jax.devices()[:n_cores]
    mesh = Mesh(np.asarray(devices), ("core",))
    in_specs = (PartitionSpec("core"),) * (n_params + n_outs)
    out_specs = (PartitionSpec("core"),) * len(out_names)
    sharded = jax.jit(
        shard_map(_body, mesh=mesh, in_specs=in_specs, out_specs=out_specs,
                  check_rep=False),
        donate_argnums=donate,
        keep_unused=True,
    )

    def execute(in_maps):
        per_core = [[np.asarray(m[nm]) for nm in in_names] for m in in_maps]
        concat_in = [
            np.concatenate([per_core[c][i] for c in range(n_cores)], axis=0)
            for i in range(n_params)
        ]
        concat_zeros = [
            np.zeros((n_cores * z.shape[0], *z.shape[1:]), z.dtype)
            for z in zero_outs
        ]
        out_arrs = sharded(*concat_in, *concat_zeros)
        out_arrs = [np.asarray(o) for o in out_arrs]
        return [
            {nm: out_arrs[i].reshape(n_cores, *out_avals[i].shape)[c]
             for i, nm in enumerate(out_names)}
            for c in range(n_cores)
        ]

    return execute


def _get_runner():
    if "runner" not in _cached:
        _cached["runner"] = _make_runner(_get_nc(), N_CORES)
    return _cached["runner"]


def _assemble(results):
    idx = np.empty((B, N, K), dtype=np.int32)
    for c in range(N_CORES):
        b = c // (N_CORES // B)
        qc = c % (N_CORES // B)
        o = np.asarray(results[c]["out"])
        idx[b, qc * QPC:(qc + 1) * QPC, :] = o.astype(np.int32)
    return idx


def run(points, k, trace=False):
    assert int(k) == K
    in_maps = _make_in_maps(points)
    last_err = None
    for attempt in range(3):
        try:
            execute = _get_runner()
            results = execute(in_maps)
            return _assemble(results), results
        except Exception as e:  # transient device wedge -> rebuild + retry
            last_err = e
            _cached.pop("runner", None)
            import time as _time
            _time.sleep(2.0 * (attempt + 1))
    raise last_err


def kernel(points, k):
    idx, _ = run(points, k)
    return idx
